# revision 14
# baseline (speedup 1.0000x reference)
"""Trainium2 Bass kernel for nn_Extractor_Processor_75368086110414.

Windowed-attention transformer block (ViTDet-style) + ResBottleneckBlock,
data-parallel over batch across 8 NeuronCores (2 images per core).

v2: fp8 (e4m3, DoubleRow) for all big GEMMs, chunk-pipelined LayerNorms
(fp32r sum-matmuls directly on f32 data), reworked softmax normalize
(DVE reciprocal on the broadcast sum PSUM), strided matmul moving
operands for the rel-pos gathers, and a software-pipelined emit order
to keep the PE dense (HAM warm).
"""

import json
import sys
import types

import numpy as np
import ml_dtypes

import concourse.bass as bass
import concourse.tile as tile
from concourse import mybir
from concourse.vector_clock import ScopedClock

F32 = mybir.dt.float32
F32R = mybir.dt.float32r
BF16 = mybir.dt.bfloat16
F8 = mybir.dt.float8e4
AF = mybir.ActivationFunctionType
OP = mybir.AluOpType
DR = mybir.MatmulPerfMode.DoubleRow

# ---------------------------------------------------------------------------
# Patch 1: the pinned walrus rejects >1 sync wait per instruction. Split the
# kernel-tail drain's waits across a chain of drains, and post-process the
# BIR JSON to peel extra waits off any instruction onto injected NoOps.
# ---------------------------------------------------------------------------
MAX_WAITS = 1
_patched = False


def _drain_and_barrier(self, tick_clock, wait_clock):
    nc = self.nc
    drain_inst = nc.sync.drain()
    wait_clock.add_sem_waits(
        drain_inst.ins, ScopedClock({None: tick_clock.global_clock})
    )
    waits = list(drain_inst.ins.sync_info.on_wait)
    if len(waits) > MAX_WAITS:
        drain_inst.ins.sync_info = mybir.SyncInfo(
            on_wait=waits[:MAX_WAITS], on_update=[]
        )
        rest = waits[MAX_WAITS:]
        for i in range(0, len(rest), MAX_WAITS):
            extra = nc.sync.drain()
            extra.ins.sync_info = mybir.SyncInfo(
                on_wait=rest[i : i + MAX_WAITS], on_update=[]
            )
    nc.all_engine_barrier()
    assert self.sems is not None
    popped = nc._tile_sem_poison_stack.pop()
    assert popped is self._sem_poison
    nc.clear_and_free_semaphores(list(self.sems.allocated().values()))
    nc.all_engine_barrier()


def _split_waits_json(data: bytes) -> bytes:
    bj = json.loads(data)
    counter = [0]
    changed = False
    for fn in bj.get("functions", []):
        for bb in fn.get("blocks", []):
            insts = bb.get("instructions")
            if not insts:
                continue
            out = []
            for inst in insts:
                si = inst.get("sync_info")
                waits = si.get("on_wait") if si else None
                if waits and len(waits) > MAX_WAITS:
                    keep = waits[-MAX_WAITS:]
                    rest = waits[:-MAX_WAITS]
                    for i in range(0, len(rest), MAX_WAITS):
                        counter[0] += 1
                        out.append({
                            "debug": inst.get("debug"),
                            "engine": inst["engine"],
                            "ins": [],
                            "name": f"I-ws{counter[0]}",
                            "opcode": "NoOp",
                            "outs": [],
                            "sync_info": {
                                "on_wait": rest[i : i + MAX_WAITS],
                                "on_update": [],
                            },
                        })
                    si["on_wait"] = keep
                    changed = True
                out.append(inst)
            bb["instructions"] = out
    if not changed:
        return data
    return json.dumps(bj).encode()


def _apply_patches():
    global _patched
    if _patched:
        return
    _patched = True
    tile.TileContext._drain_and_barrier = _drain_and_barrier
    orig = bass.Bass.to_json_bytes
    bass.Bass.to_json_bytes = lambda self, *a, **kw: _split_waits_json(
        orig(self, *a, **kw)
    )
    # Patch 2: the agent image's antenv lacks axon_hooks; register a shim so
    # run_bass_kernel_spmd(trace=True) can find the NTFF profile hook.
    if "antenv.axon_hooks" not in sys.modules:
        try:
            from trn_agent_boot.trn_boot import _ntff_profile_via_ctypes

            hook = _ntff_profile_via_ctypes("/opt/axon/libaxon_pjrt.so")
        except Exception:
            hook = None
        mod = types.ModuleType("antenv.axon_hooks")
        mod.get_axon_ntff_profile_hook = lambda: hook
        mod.set_axon_ntff_profile_hook = lambda h: None
        sys.modules["antenv.axon_hooks"] = mod


_apply_patches()

# ---------------------------------------------------------------------------
# Problem geometry (hardcoded per spec)
# ---------------------------------------------------------------------------
C = 1024
NH = 16
HD = 64
WS = 14
MLP = 4096
B, HH, WW = 16, 28, 28
NCORES = 8
BS = B // NCORES          # images per core
T = BS * HH * WW          # 1568 tokens per core
TT = 392                  # token chunk (one 14-row band of one image)
NCC = T // TT             # 4 chunks
NC_C = C // 128           # 8 c-tiles
N_WIN = BS * 4            # 8 windows per core
SW = 16.0                 # fp8 weight pre-scale
ISW = 1.0 / SW


def build_program(flags):
    has_qkvb, has_projb, has_fc2b, has_n3b = flags
    nc = bass.Bass()

    def din(name, shape, dt=F32):
        return nc.declare_dram_parameter(name, shape, dt, isOutput=False)

    xT = din("xT", [C, T])
    wqk8 = din("wqk8", [128, 8, 2 * C], F8)
    wv8 = din("wv8", [128, 8, C], F8)
    projw8 = din("projw8", [128, 8, C], F8)
    fc1wb = din("fc1wb", [C, MLP], BF16)
    fc2wb = din("fc2wb", [MLP, C], BF16)
    c1w8d = din("c1w8", [128, 8, C // 2], F8)
    c2w8d = din("c2w8", [9, 128, 4, C // 2], F8)
    c3w8d = din("c3w8", [128, 4, C], F8)
    qkvb = din("qkvb", [3 * C])
    projb = din("projb", [C])
    fc1b = din("fc1b", [MLP])
    fc2b = din("fc2b", [C])
    n1w = din("n1w", [C // 2])
    n1b = din("n1b", [C // 2])
    n2w = din("n2w", [C // 2])
    n2b = din("n2b", [C // 2])
    n3w = din("n3w", [C])
    n3b = din("n3b", [C])
    rhe = din("rhe", [128, 196], BF16)
    rwe = din("rwe", [128, 196], BF16)
    indA = din("indA", [128, 196], BF16)
    indB = din("indB", [128, 196], BF16)
    yT = nc.declare_dram_parameter("yT", [C, T], F32, isOutput=True)

    _cms = {}

    def pool(name, bufs=1, side=None):
        cm = tc.tile_pool(name=name, bufs=bufs, side=side)
        p = cm.__enter__()
        _cms[id(p)] = cm
        return p

    def pspool(name, bufs=1):
        cm = tc.tile_pool(name=name, bufs=bufs, space="PSUM")
        p = cm.__enter__()
        _cms[id(p)] = cm
        return p

    def close(*pools_):
        for p in pools_:
            _cms.pop(id(p)).__exit__(None, None, None)

    with tile.TileContext(nc) as tc:
        g = pool("glob")
        ones_bf = g.tile([128, 128], BF16, tag="ones_bf", name="ones_bf")
        nc.vector.memset(ones_bf, 1.0)
        onesf = g.tile([128, 128], F32, tag="onesf", name="onesf")
        nc.vector.memset(onesf, 1.0)
        eps5 = g.tile([128, 1], F32, tag="eps5", name="eps5")
        nc.vector.memset(eps5, 1e-5)
        eps6 = g.tile([128, 1], F32, tag="eps6", name="eps6")
        nc.vector.memset(eps6, 1e-6)

        def stage_bias(src, n, name):
            t = g.tile([128, n], F32, tag=name, name=name)
            nc.sync.dma_start(out=t, in_=src.rearrange("(o p) -> p o", p=128))
            return t

        fc1b_sb = stage_bias(fc1b, 32, "fc1b_sb")
        n1w_sb = stage_bias(n1w, 4, "n1w_sb")
        n1b_sb = stage_bias(n1b, 4, "n1b_sb")
        n2w_sb = stage_bias(n2w, 4, "n2w_sb")
        n2b_sb = stage_bias(n2b, 4, "n2b_sb")
        n3w_sb = stage_bias(n3w, 8, "n3w_sb")
        qkvb_sb = stage_bias(qkvb, 24, "qkvb_sb") if has_qkvb else None
        projb_sb = stage_bias(projb, 8, "projb_sb") if has_projb else None
        fc2b_sb = stage_bias(fc2b, 8, "fc2b_sb") if has_fc2b else None
        n3b_sb = stage_bias(n3b, 8, "n3b_sb") if has_n3b else None
        rhe_sb = g.tile([128, 196], BF16, tag="rhe_sb", name="rhe_sb")
        nc.sync.dma_start(out=rhe_sb, in_=rhe[:, :])
        rwe_sb = g.tile([128, 196], BF16, tag="rwe_sb", name="rwe_sb")
        nc.sync.dma_start(out=rwe_sb, in_=rwe[:, :])
        indA_sb = g.tile([128, 196], BF16, tag="indA_sb", name="indA_sb")
        nc.sync.dma_start(out=indA_sb, in_=indA[:, :])
        indB_sb = g.tile([128, 196], BF16, tag="indB_sb", name="indB_sb")
        nc.sync.dma_start(out=indB_sb, in_=indB[:, :])

        # ---------------- shared LN-over-channels chunk helper ----------
        def ln_chunk(sp, psp, srcs, n_ci, inv_c, eps_tile, pfx):
            """srcs: list of [128, TT] f32 APs (channel tiles for one token
            chunk). Returns (mb, rsb) [128, TT] f32 tiles, values broadcast
            on all 128 partitions via ones-matmul column sums."""
            sqs = []
            xbs = []
            for ci in range(n_ci):
                sq = sp.tile([128, TT], BF16, tag=f"{pfx}sq{ci}",
                             name=f"{pfx}sq{ci}", bufs=2)
                nc.scalar.activation(out=sq, in_=srcs[ci], func=AF.Square)
                sqs.append(sq)
                xb = sp.tile([128, TT], BF16, tag=f"{pfx}xb{ci}",
                             name=f"{pfx}xb{ci}", bufs=2)
                nc.vector.tensor_copy(out=xb, in_=srcs[ci])
                xbs.append(xb)
            ps_s = psp.tile([128, TT], F32, tag=f"{pfx}ps_s",
                            name=f"{pfx}ps_s", bufs=2)
            for ci in range(n_ci):
                nc.tensor.matmul(ps_s, ones_bf, xbs[ci],
                                 start=(ci == 0), stop=(ci == n_ci - 1))
            ps_q = psp.tile([128, TT], F32, tag=f"{pfx}ps_q",
                            name=f"{pfx}ps_q", bufs=2)
            for ci in range(n_ci):
                nc.tensor.matmul(ps_q, ones_bf, sqs[ci],
                                 start=(ci == 0), stop=(ci == n_ci - 1))
            mb = sp.tile([128, TT], F32, tag=f"{pfx}mb", name=f"{pfx}mb",
                         bufs=2)
            nc.scalar.activation(out=mb, in_=ps_s, func=AF.Copy, scale=inv_c)
            msq = sp.tile([128, TT], F32, tag=f"{pfx}msq", name=f"{pfx}msq",
                          bufs=2)
            nc.vector.tensor_mul(out=msq, in0=mb, in1=mb)
            ve = sp.tile([128, TT], F32, tag=f"{pfx}ve", name=f"{pfx}ve",
                         bufs=2)
            nc.scalar.activation(out=ve, in_=ps_q, func=AF.Copy, scale=inv_c)
            nc.vector.tensor_sub(out=ve, in0=ve, in1=msq)
            nc.scalar.activation(out=ve, in_=ve, func=AF.Ln, bias=eps_tile)
            rsb = sp.tile([128, TT], F32, tag=f"{pfx}rsb", name=f"{pfx}rsb",
                          bufs=2)
            nc.scalar.activation(out=rsb, in_=ve, func=AF.Exp, scale=-0.5)
            return mb, rsb

        # ================= Phase 1: x load, LN1, qkv (fp8 DR) ===========
        pa = pool("pa")                      # xln1_8, lives thru v-matmuls
        pb = pool("pb", side="right")        # qk tiles + v_sb, live thru attn
        p1w = pool("p1w")
        p1 = pool("p1", bufs=2)
        p1ps = pspool("p1ps")

        xln1_8 = pa.tile([128, 8, T], F8, tag="xln1_8", name="xln1_8")
        qk_sb = [pb.tile([128, T], BF16, tag=f"qk{ot}", name=f"qk{ot}")
                 for ot in range(16)]
        v_sb = pb.tile([98, N_WIN * 2 * C], BF16, tag="v_sb", name="v_sb")

        xt = {}

        def load_x_chunk(cc):
            for ci in range(NC_C):
                t = p1.tile([128, TT], F32, tag=f"xt{ci}", name=f"xt{ci}",
                            bufs=2)
                nc.sync.dma_start(
                    out=t, in_=xT[ci * 128:(ci + 1) * 128,
                                  cc * TT:(cc + 1) * TT])
                xt[(cc, ci)] = t

        load_x_chunk(0)
        wqk_sb = p1w.tile([128, 8, 2 * C], F8, tag="wqk_sb", name="wqk_sb")
        nc.sync.dma_start(out=wqk_sb, in_=wqk8[:, :, :])
        wv_sb = p1w.tile([128, 8, C], F8, tag="wv_sb", name="wv_sb")
        nc.sync.dma_start(out=wv_sb, in_=wv8[:, :, :])
        load_x_chunk(1)

        def emit_qkv_chunk(cc):
            s = slice(cc * TT, (cc + 1) * TT)
            # q, k -> 16 head-pair tiles (window-ordered cols, contiguous)
            for ot in range(16):
                ps = p1ps.tile([128, TT], F32, tag="qkps", name="qkps",
                               bufs=2)
                for j in range(4):
                    nc.tensor.matmul(
                        ps, wqk_sb[:, 2 * j:2 * j + 2,
                                   ot * 128:(ot + 1) * 128],
                        xln1_8[:, 2 * j:2 * j + 2, s],
                        start=(j == 0), stop=(j == 3), perf_mode=DR)
                dst = qk_sb[ot][:, s]
                if ot % 2 == 0:
                    if has_qkvb:
                        nc.scalar.activation(
                            out=dst, in_=ps, func=AF.Identity, scale=ISW,
                            bias=qkvb_sb[:, ot:ot + 1])
                    else:
                        nc.scalar.activation(out=dst, in_=ps, func=AF.Copy,
                                             scale=ISW)
                else:
                    if has_qkvb:
                        nc.vector.tensor_scalar(
                            out=dst, in0=ps, scalar1=ISW,
                            scalar2=qkvb_sb[:, ot:ot + 1],
                            op0=OP.mult, op1=OP.add)
                    else:
                        nc.vector.tensor_scalar(
                            out=dst, in0=ps, scalar1=ISW, scalar2=None,
                            op0=OP.mult)
            # v: stationary = xln1 window-half chunks, moving = wv
            for g4 in range(4):            # (ww, u) half within chunk
                base = cc * TT + g4 * 98
                vcol = (cc * 2 + (g4 // 2)) * 2 * C + (g4 % 2) * C
                for sl in range(2):
                    pv = p1ps.tile([98, 512], F32, tag="vps", name="vps",
                                   bufs=2)
                    for j in range(4):
                        nc.tensor.matmul(
                            pv, xln1_8[:, 2 * j:2 * j + 2, base:base + 98],
                            wv_sb[:, 2 * j:2 * j + 2,
                                  sl * 512:(sl + 1) * 512],
                            start=(j == 0), stop=(j == 3), perf_mode=DR)
                    dst = v_sb[:, vcol + sl * 512:vcol + (sl + 1) * 512]
                    if (g4 + sl) % 2 == 0:
                        nc.scalar.activation(out=dst, in_=pv, func=AF.Copy,
                                             scale=ISW)
                    else:
                        nc.vector.tensor_scalar(
                            out=dst, in0=pv, scalar1=ISW, scalar2=None,
                            op0=OP.mult)

        for cc in range(NCC + 1):
            if cc < NCC:
                if cc + 2 < NCC:
                    load_x_chunk(cc + 2)
                srcs = [xt[(cc, ci)] for ci in range(NC_C)]
                mb, rsb = ln_chunk(p1, p1ps, srcs, NC_C, 1.0 / C, eps5, "l1")
                # apply + cast to fp8, reorder row-major -> window-ordered
                for ci in range(NC_C):
                    tmp = p1.tile([128, TT], F32, tag="l1tmp", name="l1tmp",
                                  bufs=3)
                    nc.vector.tensor_sub(out=tmp, in0=srcs[ci], in1=mb)
                    tv = tmp.rearrange("p (a w b) -> p w a b", a=WS, w=2)
                    rv = rsb.rearrange("p (a w b) -> p w a b", a=WS, w=2)
                    xo = xln1_8[:, ci, cc * TT:(cc + 1) * TT].rearrange(
                        "p (w a b) -> p w a b", w=2, a=WS)
                    for ww in range(2):
                        nc.vector.tensor_mul(out=xo[:, ww], in0=tv[:, ww],
                                             in1=rv[:, ww])
            if cc >= 1:
                emit_qkv_chunk(cc - 1)
        close(p1ps, p1, p1w, pa)

        # ================= Phase 2: windowed attention ==================
        pc = pool("pc")                       # xat8, lives thru proj
        p3w = pool("p3w")
        pres = pool("pres")                   # xre residual, lives thru P3
        p2 = pool("p2", bufs=2)
        p2ps = pspool("p2ps")
        xat8 = pc.tile([128, 8, T], F8, tag="xat8", name="xat8")
        projw_sb = p3w.tile([128, 8, C], F8, tag="projw_sb", name="projw_sb")
        nc.sync.dma_start(out=projw_sb, in_=projw8[:, :, :])
        xre = {}
        for cc in range(NCC):
            for ci in range(NC_C):
                t = pres.tile([128, TT], F32, tag=f"xre{ci}",
                              name=f"xre{ci}", bufs=4)
                nc.sync.dma_start(
                    out=t, in_=xT[ci * 128:(ci + 1) * 128,
                                  cc * TT:(cc + 1) * TT])
                xre[(cc, ci)] = t

        rels = {}

        def emit_rel(hp, idxs):
            if hp >= 8:
                return
            if hp not in rels:
                rels[hp] = p2.tile([128, T], BF16, tag="rel", name="rel",
                                   bufs=2)
            relt = rels[hp]
            qv = qk_sb[hp].rearrange("p (w a b) -> p w a b", w=N_WIN, a=WS)
            rv3 = relt.rearrange("p (w a b) -> p w a b", w=N_WIN, a=WS)
            for idx in idxs:
                rp = p2ps.tile([128, 112], F32, tag="rp", name="rp", bufs=2)
                nc.tensor.matmul(
                    rp[0:14, :], rhe_sb[0:64, idx * 14:(idx + 1) * 14],
                    qv[0:64, :, idx, :], start=True, stop=True,
                    tile_position=(0, 0))
                nc.tensor.matmul(
                    rp[32:46, :], rhe_sb[64:128, idx * 14:(idx + 1) * 14],
                    qv[64:128, :, idx, :], start=True, stop=True,
                    tile_position=(64, 32))
                nc.tensor.matmul(
                    rp[64:78, :], rwe_sb[0:64, idx * 14:(idx + 1) * 14],
                    qv[0:64, :, :, idx], start=True, stop=True,
                    tile_position=(0, 64))
                nc.tensor.matmul(
                    rp[96:110, :], rwe_sb[64:128, idx * 14:(idx + 1) * 14],
                    qv[64:128, :, :, idx], start=True, stop=True,
                    tile_position=(64, 96))
                rp3 = rp.rearrange("p (w b) -> p w b", w=N_WIN)
                nc.vector.tensor_copy(out=rv3[0:46, :, idx, :],
                                      in_=rp3[0:46, :, :])
                nc.vector.tensor_copy(out=rv3[64:110, :, :, idx],
                                      in_=rp3[64:110, :, :])

        def rm_win_view(t3, hp, w):
            # row-major [128, 14, 14] view of window w in xat8[:, hp, :]
            i, wh, ww = w // 4, (w // 2) % 2, w % 2
            r = t3[:, hp, :].rearrange("p (i a b) -> p i a b", i=BS, a=28)
            return r[:, i, wh * 14:(wh + 1) * 14, ww * 14:(ww + 1) * 14]

        emit_rel(0, range(WS))
        for hp in range(8):
            qT = qk_sb[hp]
            kT = qk_sb[8 + hp]
            relt = rels.pop(hp)
            ptss = {}
            pvs = {}

            def emit_scores(w):
                for u in range(2):
                    st = p2ps.tile([98, 392], F32, tag="st", name="st",
                                   bufs=2)
                    nc.tensor.matmul(
                        st[:, 0:196],
                        kT[0:64, w * 196 + u * 98:w * 196 + (u + 1) * 98],
                        qT[0:64, w * 196:(w + 1) * 196],
                        start=True, stop=False, tile_position=(0, 0))
                    nc.tensor.matmul(
                        st[:, 0:196], indA_sb[0:110, u * 98:(u + 1) * 98],
                        relt[0:110, w * 196:(w + 1) * 196],
                        start=False, stop=True, tile_position=(0, 0))
                    nc.tensor.matmul(
                        st[:, 196:392],
                        kT[64:128, w * 196 + u * 98:w * 196 + (u + 1) * 98],
                        qT[64:128, w * 196:(w + 1) * 196],
                        start=True, stop=False, tile_position=(64, 0))
                    nc.tensor.matmul(
                        st[:, 196:392], indB_sb[0:110, u * 98:(u + 1) * 98],
                        relt[0:110, w * 196:(w + 1) * 196],
                        start=False, stop=True, tile_position=(0, 0))
                    pts = p2.tile([98, 392], BF16, tag="pts", name="pts",
                                  bufs=6)
                    nc.scalar.activation(out=pts, in_=st, func=AF.Exp)
                    ptss[(w, u)] = pts

            def emit_pv(w):
                sm = p2ps.tile([128, 392], F32, tag="sm", name="sm", bufs=2)
                for u in range(2):
                    nc.tensor.matmul(sm, ones_bf[0:98, :], ptss[(w, u)],
                                     start=(u == 0), stop=(u == 1))
                lt = p2.tile([128, 392], F32, tag="lt", name="lt", bufs=2)
                nc.scalar.activation(out=lt, in_=sm, func=AF.Ln)
                rbbs = p2.tile([128, 392], F32, tag="rbbs", name="rbbs",
                               bufs=2)
                nc.scalar.activation(out=rbbs, in_=lt, func=AF.Exp,
                                     scale=-1.0)
                pv = p2ps.tile([128, 196], F32, tag="pv", name="pv", bufs=2)
                for u in range(2):
                    vcol = (w * 2 + u) * C + hp * 128
                    pts = ptss.pop((w, u))
                    nc.tensor.matmul(
                        pv[0:64, :], v_sb[:, vcol:vcol + 64],
                        pts[:, 0:196],
                        start=(u == 0), stop=(u == 1), tile_position=(0, 0))
                    nc.tensor.matmul(
                        pv[64:128, :], v_sb[:, vcol + 64:vcol + 128],
                        pts[:, 196:392],
                        start=(u == 0), stop=(u == 1), tile_position=(0, 64))
                xv = rm_win_view(xat8, hp, w)
                rbA = rbbs[0:64, 0:196].rearrange("p (a b) -> p a b", a=WS)
                rbB = rbbs[64:128, 196:392].rearrange("p (a b) -> p a b",
                                                      a=WS)
                pvv = pv.rearrange("p (a b) -> p a b", a=WS)
                nc.vector.tensor_mul(out=xv[0:64], in0=pvv[0:64], in1=rbA)
                nc.vector.tensor_mul(out=xv[64:128], in0=pvv[64:128],
                                     in1=rbB)

            for w in range(N_WIN):
                emit_scores(w)
                if w == 3:
                    emit_rel(hp + 1, range(0, 7))
                if w == 5:
                    emit_rel(hp + 1, range(7, WS))
                if w >= 2:
                    emit_pv(w - 2)
            emit_pv(N_WIN - 2)
            emit_pv(N_WIN - 1)
        close(p2ps, p2, pb)

        # ================= Phase 3: proj + residual + LN2 ===============
        px2 = pool("px2", side="right")       # x2, lives to the end
        p4x = pool("p4x", side="right")       # xln2_8, lives thru fc1
        p3 = pool("p3", bufs=2)
        p3ps = pspool("p3ps")
        x2 = [px2.tile([128, T], F32, tag=f"x2_{ot}", name=f"x2_{ot}")
              for ot in range(NC_C)]
        xln2b = p4x.tile([128, 8, T], BF16, tag="xln2b", name="xln2b")
        for cc in range(NCC):
            s = slice(cc * TT, (cc + 1) * TT)
            for ot in range(NC_C):
                ps = p3ps.tile([128, TT], F32, tag="p3ps", name="p3ps",
                               bufs=3)
                for j in range(4):
                    nc.tensor.matmul(
                        ps, projw_sb[:, 2 * j:2 * j + 2,
                                     ot * 128:(ot + 1) * 128],
                        xat8[:, 2 * j:2 * j + 2, s],
                        start=(j == 0), stop=(j == 3), perf_mode=DR)
                if has_projb:
                    tmp = p3.tile([128, TT], BF16, tag="prtmp", name="prtmp",
                                  bufs=2)
                    nc.scalar.activation(out=tmp, in_=ps, func=AF.Identity,
                                         scale=ISW,
                                         bias=projb_sb[:, ot:ot + 1])
                    nc.vector.tensor_add(out=x2[ot][:, s], in0=tmp,
                                         in1=xre[(cc, ot)])
                else:
                    nc.vector.scalar_tensor_tensor(
                        out=x2[ot][:, s], in0=ps, scalar=ISW,
                        in1=xre[(cc, ot)], op0=OP.mult, op1=OP.add)
            srcs = [x2[ci][:, s] for ci in range(NC_C)]
            mb, rsb = ln_chunk(p3, p3ps, srcs, NC_C, 1.0 / C, eps5, "l2")
            for ci in range(NC_C):
                tmp = p3.tile([128, TT], F32, tag="l2tmp", name="l2tmp",
                              bufs=3)
                nc.vector.tensor_sub(out=tmp, in0=x2[ci][:, s], in1=mb)
                nc.vector.tensor_mul(out=xln2b[:, ci, s], in0=tmp, in1=rsb)
        close(p3ps, p3, pres, p3w, pc)

        # ================= Phase 4: MLP (fp8 DR) ========================
        p5w = pool("p5w")                     # conv weights
        p5x = pool("p5x")                     # x3_8, lives thru conv1
        pr1 = pool("pr1")                     # r1pad8, lives thru conv2
        p4 = pool("p4", bufs=2)
        p4ps = pspool("p4ps")
        x3_8 = p5x.tile([128, 8, T], F8, tag="x3_8", name="x3_8")
        r1pad8 = pr1.tile([128, 4, BS * 900 + 2], F8, tag="r1pad8",
                          name="r1pad8")
        nc.gpsimd.memset(r1pad8, 0.0)
        c1w_sb = p5w.tile([128, 8, C // 2], F8, tag="c1w_sb", name="c1w_sb")
        nc.sync.dma_start(out=c1w_sb, in_=c1w8d[:, :, :])
        c2w_sb = []
        for tap in range(9):
            t = p5w.tile([128, 4, C // 2], F8, tag=f"c2w_{tap}",
                         name=f"c2w_{tap}")
            nc.sync.dma_start(out=t, in_=c2w8d[tap, :, :, :])
            c2w_sb.append(t)
        c3w_sb = p5w.tile([128, 4, C], F8, tag="c3w_sb", name="c3w_sb")
        nc.sync.dma_start(out=c3w_sb, in_=c3w8d[:, :, :])

        fc1v = fc1wb.rearrange("(k p) m -> p k m", p=128)
        fc2v = fc2wb.rearrange("(k p) m -> p k m", p=128)
        for tp in range(2):
            hb = p4.tile([128, 32, 784], BF16, tag="hbuf", name="hbuf",
                         bufs=1)
            for og in range(8):
                w1t = p4.tile([128, 8, 512], BF16, tag="w1", name="w1",
                              bufs=2)
                nc.sync.dma_start(out=w1t,
                                  in_=fc1v[:, :, og * 512:(og + 1) * 512])
                for otl in range(4):
                    ki = og * 4 + otl
                    for tl in range(2):
                        t0 = tp * 784 + tl * TT
                        ps = p4ps.tile([128, TT], F32, tag="fc1ps",
                                       name="fc1ps", bufs=3)
                        for j in range(8):
                            nc.tensor.matmul(
                                ps, w1t[:, j, otl * 128:(otl + 1) * 128],
                                xln2b[:, j, t0:t0 + TT],
                                start=(j == 0), stop=(j == 7))
                        nc.scalar.activation(
                            out=hb[:, ki, tl * TT:(tl + 1) * TT], in_=ps,
                            func=AF.Gelu, bias=fc1b_sb[:, ki:ki + 1])
            for og2 in range(8):
                w2t = p4.tile([128, 32, 128], BF16, tag="w2", name="w2",
                              bufs=2)
                nc.sync.dma_start(out=w2t,
                                  in_=fc2v[:, :, og2 * 128:(og2 + 1) * 128])
                ot = og2
                for tl in range(2):
                    t0 = tp * 784 + tl * TT
                    ps = p4ps.tile([128, TT], F32, tag="fc2ps",
                                   name="fc2ps", bufs=3)
                    for k in range(32):
                        nc.tensor.matmul(
                            ps, w2t[:, k, :],
                            hb[:, k, tl * TT:(tl + 1) * TT],
                            start=(k == 0), stop=(k == 31))
                    if has_fc2b:
                        tmp = p4.tile([128, TT], BF16, tag="f2tmp",
                                      name="f2tmp", bufs=2)
                        nc.scalar.activation(
                            out=tmp, in_=ps, func=AF.Identity,
                            bias=fc2b_sb[:, ot:ot + 1])
                        nc.vector.tensor_add(
                            out=x2[ot][:, t0:t0 + TT], in0=tmp,
                            in1=x2[ot][:, t0:t0 + TT])
                    else:
                        nc.vector.tensor_add(
                            out=x2[ot][:, t0:t0 + TT], in0=ps,
                            in1=x2[ot][:, t0:t0 + TT])
            # cast this half's x2 -> fp8 conv input
            hs = slice(tp * 784, (tp + 1) * 784)
            for ci in range(NC_C):
                if ci % 2 == 0:
                    nc.scalar.copy(out=x3_8[:, ci, hs], in_=x2[ci][:, hs])
                else:
                    nc.vector.tensor_copy(out=x3_8[:, ci, hs],
                                          in_=x2[ci][:, hs])
        close(p4ps, p4, p4x)

        # ================= Phase 5: ResBottleneckBlock (fp8 DR) =========
        p5a = pool("p5a", bufs=2)
        p5aps = pspool("p5aps")
        c1s = [p5a.tile([128, T], F32, tag=f"c1s{ot}", name=f"c1s{ot}",
                        bufs=1) for ot in range(4)]
        r1v = r1pad8[:, :, 0:1800].rearrange("p c (i y x) -> p c i y x",
                                             i=BS, y=30)

        for cc in range(NCC):
            s = slice(cc * TT, (cc + 1) * TT)
            i, wh = cc // 2, cc % 2
            for ot in range(4):
                ps = p5aps.tile([128, TT], F32, tag="c1ps", name="c1ps",
                               bufs=2)
                for j in range(4):
                    nc.tensor.matmul(
                        ps, c1w_sb[:, 2 * j:2 * j + 2,
                                   ot * 128:(ot + 1) * 128],
                        x3_8[:, 2 * j:2 * j + 2, s],
                        start=(j == 0), stop=(j == 3), perf_mode=DR)
                nc.scalar.activation(out=c1s[ot][:, s], in_=ps, func=AF.Copy,
                                     scale=ISW)
            srcs = [c1s[ci][:, s] for ci in range(4)]
            mb, rsb = ln_chunk(p5a, p5aps, srcs, 4, 1.0 / 512, eps6, "n1")
            for ci in range(4):
                tmp = p5a.tile([128, TT], F32, tag="n1tmp", name="n1tmp",
                               bufs=3)
                nc.vector.tensor_sub(out=tmp, in0=c1s[ci][:, s], in1=mb)
                nc.vector.tensor_mul(out=tmp, in0=tmp, in1=rsb)
                nc.scalar.activation(
                    out=r1v[:, ci, i, 1 + 14 * wh:15 + 14 * wh, 1:29],
                    in_=tmp.rearrange("p (a b) -> p a b", a=WS),
                    func=AF.Gelu, scale=n1w_sb[:, ci:ci + 1],
                    bias=n1b_sb[:, ci:ci + 1])
        close(p5aps, p5a)

        p5b = pool("p5b", bufs=2)
        p5bps = pspool("p5bps")
        pr2 = pool("pr2", side="right")
        r2_8 = pr2.tile([128, 4, T], F8, tag="r2_8", name="r2_8")
        c2s = [p5b.tile([128, T], F32, tag=f"c2s{ot}", name=f"c2s{ot}",
                        bufs=1) for ot in range(4)]
        for cc in range(NCC):
            s = slice(cc * TT, (cc + 1) * TT)
            i, wh = cc // 2, cc % 2
            for ot in range(4):
                ps = p5bps.tile([128, 420], F32, tag="c2ps", name="c2ps",
                               bufs=2)
                n = 0
                for tap in range(9):
                    dy, dx = tap // 3, tap % 3
                    st0 = i * 900 + (14 * wh + dy) * 30 + dx
                    for j in range(2):
                        nc.tensor.matmul(
                            ps, c2w_sb[tap][:, 2 * j:2 * j + 2,
                                            ot * 128:(ot + 1) * 128],
                            r1pad8[:, 2 * j:2 * j + 2, st0:st0 + 420],
                            start=(n == 0), stop=(n == 17), perf_mode=DR)
                        n += 1
                nc.scalar.activation(
                    out=c2s[ot][:, s].rearrange("p (h x) -> p h x", h=WS),
                    in_=ps.rearrange("p (h x) -> p h x", h=WS)[:, :, 0:28],
                    func=AF.Copy, scale=ISW)
            srcs = [c2s[ci][:, s] for ci in range(4)]
            mb, rsb = ln_chunk(p5b, p5bps, srcs, 4, 1.0 / 512, eps6, "n2")
            for ci in range(4):
                tmp = p5b.tile([128, TT], F32, tag="n2tmp", name="n2tmp",
                               bufs=3)
                nc.vector.tensor_sub(out=tmp, in0=c2s[ci][:, s], in1=mb)
                nc.vector.tensor_mul(out=tmp, in0=tmp, in1=rsb)
                nc.scalar.activation(
                    out=r2_8[:, ci, s], in_=tmp, func=AF.Gelu,
                    scale=n2w_sb[:, ci:ci + 1], bias=n2b_sb[:, ci:ci + 1])
        close(p5bps, p5b, pr1, p5x)

        p5c = pool("p5c", bufs=2)
        p5cps = pspool("p5cps")
        for cc in range(NCC):
            s = slice(cc * TT, (cc + 1) * TT)
            c3c = []
            for ot in range(NC_C):
                ps = p5cps.tile([128, TT], F32, tag="c3ps", name="c3ps",
                               bufs=2)
                for j in range(2):
                    nc.tensor.matmul(
                        ps, c3w_sb[:, 2 * j:2 * j + 2,
                                   ot * 128:(ot + 1) * 128],
                        r2_8[:, 2 * j:2 * j + 2, s],
                        start=(j == 0), stop=(j == 1), perf_mode=DR)
                t = p5c.tile([128, TT], F32, tag=f"c3s{ot}",
                             name=f"c3s{ot}", bufs=2)
                nc.scalar.activation(out=t, in_=ps, func=AF.Copy, scale=ISW)
                c3c.append(t)
            mb, rsb = ln_chunk(p5c, p5cps, c3c, NC_C, 1.0 / C, eps6, "n3")
            for ot in range(NC_C):
                tmp = p5c.tile([128, TT], F32, tag="n3tmp", name="n3tmp",
                               bufs=3)
                nc.vector.tensor_sub(out=tmp, in0=c3c[ot], in1=mb)
                nc.vector.tensor_mul(out=tmp, in0=tmp, in1=rsb)
                yt = p5c.tile([128, TT], F32, tag="yt", name="yt", bufs=3)
                if has_n3b:
                    nc.vector.tensor_scalar(
                        out=tmp, in0=tmp, scalar1=n3w_sb[:, ot:ot + 1],
                        scalar2=n3b_sb[:, ot:ot + 1], op0=OP.mult,
                        op1=OP.add)
                    nc.vector.tensor_add(out=yt, in0=tmp, in1=x2[ot][:, s])
                else:
                    nc.vector.scalar_tensor_tensor(
                        out=yt, in0=tmp, scalar=n3w_sb[:, ot:ot + 1],
                        in1=x2[ot][:, s], op0=OP.mult, op1=OP.add)
                nc.sync.dma_start(out=yT[ot * 128:(ot + 1) * 128, s], in_=yt)
        close(p5cps, p5c, pr2, p5w, px2, g)

    return nc


# ---------------------------------------------------------------------------
# Host side
# ---------------------------------------------------------------------------
_program_cache = {}


def _get_program(flags):
    if flags not in _program_cache:
        _program_cache[flags] = build_program(flags)
    return _program_cache[flags]


def _f8(x):
    return np.ascontiguousarray(
        np.clip(x * SW, -240.0, 240.0)).astype(ml_dtypes.float8_e4m3)


def _bf(x):
    return np.ascontiguousarray(x).astype(ml_dtypes.bfloat16)


def _pack8(wT, nk, m):
    # wT [K, M] fp32 -> [128, nk, m] fp8 (pre-scaled by SW)
    a = np.asarray(wT, np.float32).reshape(nk, 128, m).transpose(1, 0, 2)
    return _f8(a)


def prep_inputs(inputs):
    f = {k: np.asarray(v, dtype=np.float32) for k, v in inputs.items()}
    scale = HD ** -0.5

    qkv_w = f["qkv_w"].copy()          # [3C, C]
    qkv_b = f["qkv_b"].copy()          # [3C]
    qkv_w[:C] *= scale                 # fold 1/sqrt(hd) into q
    qkv_b[:C] *= scale
    # fold ln1 affine into qkv
    qkv_wT = (qkv_w * f["ln1_w"][None, :]).T.copy()      # [C, 3C]
    qkv_b_eff = qkv_b + qkv_w @ f["ln1_b"]
    # fold v-bias into proj bias (softmax weights sum to 1)
    proj_b_eff = f["proj_b"] + f["proj_w"] @ qkv_b_eff[2 * C:]
    # fold ln2 affine into fc1
    fc1_wT = (f["fc1_w"] * f["ln2_w"][None, :]).T.copy()  # [C, MLP]
    fc1_b_eff = f["fc1_b"] + f["fc1_w"] @ f["ln2_b"]

    proj_wT = f["proj_w"].T.copy()
    fc2_wT = f["fc2_w"].T.copy()
    c1_wT = f["conv1_w"][:, :, 0, 0].T.copy()            # [C, C/2]
    c2 = f["conv2_w"]                                    # [O, I, 3, 3]
    c2_wT = np.ascontiguousarray(
        c2.transpose(2, 3, 1, 0).reshape(9, C // 2, C // 2))
    c3_wT = f["conv3_w"][:, :, 0, 0].T.copy()            # [C/2, C]

    # rel pos tables: rhe[p, qh*14+kh] = 8*rel_pos_h[qh-kh+13, p%64]
    rh8 = 8.0 * f["rel_pos_h"]                           # [27, 64]
    rw8 = 8.0 * f["rel_pos_w"]
    qh_i, kh_i = np.meshgrid(np.arange(WS), np.arange(WS), indexing="ij")
    idx = qh_i - kh_i + WS - 1                           # [qh, kh]
    rhe = rh8[idx]                                       # [qh, kh, 64]
    rwe = rw8[idx]
    rhe_t = np.zeros((128, 196), np.float32)
    rwe_t = np.zeros((128, 196), np.float32)
    rhe_flat = rhe.transpose(2, 0, 1).reshape(64, 196)   # [c, qh*14+kh]
    rwe_flat = rwe.transpose(2, 0, 1).reshape(64, 196)
    rhe_t[0:64] = rhe_flat
    rhe_t[64:128] = rhe_flat
    rwe_t[0:64] = rwe_flat
    rwe_t[64:128] = rwe_flat

    # indicators vs rel rows: A-kh 0:14, B-kh 32:46, A-kw 64:78, B-kw 96:110
    indA = np.zeros((128, 196), np.float32)
    indB = np.zeros((128, 196), np.float32)
    kt = np.arange(196)
    for j in range(WS):
        indA[j, kt // 14 == j] = 1.0
        indA[64 + j, kt % 14 == j] = 1.0
        indB[32 + j, kt // 14 == j] = 1.0
        indB[96 + j, kt % 14 == j] = 1.0

    flags = (
        bool(np.any(qkv_b_eff[:2 * C] != 0.0)),
        bool(np.any(proj_b_eff != 0.0)),
        bool(np.any(f["fc2_b"] != 0.0)),
        bool(np.any(f["n3_b"] != 0.0)),
    )

    common = {
        "wqk8": _pack8(qkv_wT[:, 0:2 * C], 8, 2 * C),
        "wv8": _pack8(qkv_wT[:, 2 * C:3 * C], 8, C),
        "projw8": _pack8(proj_wT, 8, C),
        "fc1wb": _bf(fc1_wT),
        "fc2wb": _bf(fc2_wT),
        "c1w8": _pack8(c1_wT, 8, C // 2),
        "c2w8": np.stack([_pack8(c2_wT[t], 4, C // 2) for t in range(9)]),
        "c3w8": _pack8(c3_wT, 4, C),
        "qkvb": qkv_b_eff.astype(np.float32),
        "projb": proj_b_eff.astype(np.float32),
        "fc1b": fc1_b_eff.astype(np.float32),
        "fc2b": f["fc2_b"],
        "n1w": f["n1_w"], "n1b": f["n1_b"],
        "n2w": f["n2_w"], "n2b": f["n2_b"],
        "n3w": f["n3_w"], "n3b": f["n3_b"],
        "rhe": _bf(rhe_t), "rwe": _bf(rwe_t),
        "indA": _bf(indA), "indB": _bf(indB),
    }
    x = f["x"]                                           # [B, 28, 28, C]
    in_maps = []
    for core in range(NCORES):
        xs = x[core * BS:(core + 1) * BS].reshape(T, C).T  # [C, T]
        m = dict(common)
        m["xT"] = np.ascontiguousarray(xs)
        in_maps.append(m)
    return in_maps, flags


def run(inputs, trace=False):
    from concourse.bass_utils import run_bass_kernel_spmd

    in_maps, flags = prep_inputs(inputs)
    nc = _get_program(flags)
    res = run_bass_kernel_spmd(nc, in_maps, core_ids=list(range(NCORES)),
                               trace=trace)
    outs = []
    for core in range(NCORES):
        yt = res.results[core]["yT"]                     # [C, T]
        outs.append(yt.T.reshape(BS, HH, WW, C))
    y = np.concatenate(outs, axis=0).astype(np.float32)
    return y, res


def kernel(**inputs):
    y, _ = run(inputs, trace=False)
    return y
